# revision 100
# baseline (speedup 1.0000x reference)
"""Causal MHA with RoPE on 8 Trainium2 NeuronCores.

Sharding: tensor-parallel over heads. Core c owns heads {2c, 2c+1} (a 128-wide
slice of the model dim). Each core computes Q/K/V projections for its heads,
full causal attention, and a partial o_proj; the host sums the 8 partial
outputs (the "all-reduce").

Schedule: a software-pipelined flat stream of "steps", one per (batch, query
chunk), emitted so the in-order PE stream rarely parks on a dependency:
  - x is pre-transposed on the host ([128, 8, BS] bf16); chunk g+3 prefetched
  - V projection of chunk g and Q/K projection + RoPE of chunk g+1 are
    emitted as small matmul units through a fill queue
  - RoPE uses a DVE stream_shuffle (pair partner = +16 within each 32-row
    quadrant, set up by the host-side head-dim permutation) + cos/sin
    multiplies, with the final add on gpsimd
  - attention for chunk g: per 256-key tile: scores (PE, causal-trimmed at
    128-col granularity) -> exp (ACT) -> diagonal triangle mask (gpsimd)
  - per (query tile, head): one PV chain unit — a single start->stop PSUM
    accumulation (one open group per 2KB PSUM bank at a time!) in [q, d]
    form with a ones column for the softmax denominator, then reciprocal
    (DVE) and a normalize fused into the PSUM->SBUF copy (tensor_scalar_mul)
  - per finished query tile: PE-transpose of O, o_proj, bf16 y store; these
    units are interleaved as further PE fill
PSUM budget (8 banks): scores 2x[128,1024] (4) + proj work ring (2) +
pv-chain/transpose/o_proj ring (2).
"""
import sys
sys.path.insert(0, '/opt/trn_rl_repo')

from collections import deque

import numpy as np
import ml_dtypes

import concourse.bass as bass
from concourse import bacc
import concourse.mybir as mybir
import concourse.tile as tile
from concourse.bass_utils import run_bass_kernel_spmd

BFNP = ml_dtypes.bfloat16
F32 = mybir.dt.float32
BF16 = mybir.dt.bfloat16
AF = mybir.ActivationFunctionType
ALU = mybir.AluOpType

B, S, D = 4, 2048, 1024
NCORES = 8
BS = B * S
ROPE_THETA = 10000.0

# engine knobs (fallbacks if gpsimd rejects tensor ops: set to False)
POOL_MASK = True
POOL_ROPE_ADD = True
USE_SHUFFLE = True
DVE_DIVIDE = False
SHUF_MASK = [(i + 16) % 32 for i in range(32)]

TRACE = False
LAST_RESULTS = None


def build_nc(nb=B, debug=False):
    nc = bacc.Bacc()
    xb = nc.dram_tensor("xb", [128, 8, BS], BF16, kind="ExternalInput")
    wqt = nc.dram_tensor("wqt", [128, D], BF16, kind="ExternalInput")
    wkt = nc.dram_tensor("wkt", [128, D], BF16, kind="ExternalInput")
    wvt = nc.dram_tensor("wvt", [128, D], BF16, kind="ExternalInput")
    wot = nc.dram_tensor("wot", [128, D], BF16, kind="ExternalInput")
    cosb = nc.dram_tensor("cosb", [128, S], BF16, kind="ExternalInput")
    sinf = nc.dram_tensor("sinf", [128, S], BF16, kind="ExternalInput")
    pswap = nc.dram_tensor("pswap", [128, 128], BF16, kind="ExternalInput")
    tri = nc.dram_tensor("tri", [128, 128], BF16, kind="ExternalInput")
    iden = nc.dram_tensor("iden", [128, 128], BF16, kind="ExternalInput")
    y = nc.dram_tensor("y", [BS, D], BF16, kind="ExternalOutput")

    NCHUNK = 4 * nb

    with tile.TileContext(nc) as tc:
        with tc.tile_pool(name="const", bufs=1) as constp, \
             tc.tile_pool(name="xt", bufs=4) as xtp, \
             tc.tile_pool(name="qk", bufs=2) as qkp, \
             tc.tile_pool(name="vsb", bufs=2) as vsp, \
             tc.tile_pool(name="rope", bufs=6) as ropep, \
             tc.tile_pool(name="ptile", bufs=28) as pp, \
             tc.tile_pool(name="osb", bufs=2) as osp, \
             tc.tile_pool(name="rc", bufs=2) as rcp, \
             tc.tile_pool(name="ott", bufs=4) as otp, \
             tc.tile_pool(name="yout", bufs=6) as yop, \
             tc.tile_pool(name="wk", bufs=2, space="PSUM") as wkp, \
             tc.tile_pool(name="stp", bufs=2, space="PSUM") as stp, \
             tc.tile_pool(name="pvp", bufs=2, space="PSUM") as pvp:
            xt_of = {}

            def xt_prefetch(c):
                if c < NCHUNK and c not in xt_of:
                    bc, c4 = divmod(c, 4)
                    t0 = (bc % B) * S + 512 * c4
                    xt = xtp.tile([128, 8, 512], BF16, tag="xt", name=f"xt{c}")
                    nc.sync.dma_start(out=xt, in_=xb[:, :, t0:t0 + 512])
                    xt_of[c] = xt

            xt_prefetch(0)
            wtile = constp.tile([128, 2], F32)
            nc.gpsimd.memset(wtile, 0.0)
            warm = constp.tile([128, 2], F32)
            nc.scalar.activation(warm, wtile, AF.Exp)
            # PE pstate warm-up: zero matmuls while the first x chunk loads
            wz = constp.tile([128, 128], BF16)
            nc.gpsimd.memset(wz, 0.0)
            wmm = wkp.tile([128, 128], F32, tag="wk", name="wmm")
            for _ in range(48):
                nc.tensor.matmul(wmm, wz, wz, start=True, stop=True)
            wq_sb = constp.tile([128, 8, 128], BF16)
            nc.scalar.dma_start(out=wq_sb, in_=wqt.rearrange("p (a f) -> p a f", a=8))
            wk_sb = constp.tile([128, 8, 128], BF16)
            nc.scalar.dma_start(out=wk_sb, in_=wkt.rearrange("p (a f) -> p a f", a=8))
            xt_prefetch(1)
            xt_prefetch(2)
            wv_sb = constp.tile([128, 8, 128], BF16)
            nc.scalar.dma_start(out=wv_sb, in_=wvt.rearrange("p (a f) -> p a f", a=8))
            wot_sb = constp.tile([128, D], BF16)
            nc.scalar.dma_start(out=wot_sb, in_=wot[:, :])
            cos_sb = constp.tile([128, S], BF16)
            nc.scalar.dma_start(out=cos_sb, in_=cosb[:, :])
            sin_sb = constp.tile([128, S], BF16)
            nc.scalar.dma_start(out=sin_sb, in_=sinf[:, :])
            if not USE_SHUFFLE:
                psw_sb = constp.tile([128, 128], BF16)
                nc.gpsimd.dma_start(out=psw_sb, in_=pswap[:, :])
            tri_sb = constp.tile([128, 128], BF16)
            nc.gpsimd.dma_start(out=tri_sb, in_=tri[:, :])
            id_sb = constp.tile([128, 128], BF16)
            nc.gpsimd.dma_start(out=id_sb, in_=iden[:, :])
            # ---- per-batch SBUF tile state ----
            qtr_of, ktr_of, v_of, o_of, rc_of = {}, {}, {}, {}, {}

            def batch_tiles(bi):
                if bi not in qtr_of:
                    qtr_of[bi] = qkp.tile([128, S], BF16, tag="qtr",
                                          name=f"qtr{bi}")
                    ktr_of[bi] = qkp.tile([128, S], BF16, tag="ktr",
                                          name=f"ktr{bi}")
                    v_of[bi] = vsp.tile([128, 16, 130], BF16, tag="v",
                                        name=f"v{bi}")
                    nc.gpsimd.memset(v_of[bi][:, :, 64:65], 1.0)
                    nc.gpsimd.memset(v_of[bi][:, :, 129:130], 1.0)
                    o_of[bi] = osp.tile([128, 16, 128], BF16, tag="o",
                                        name=f"o{bi}")
                    rc_of[bi] = rcp.tile([128, 32], F32, tag="rcp",
                                         name=f"rc{bi}")

            # ---- fill queue: callables emitting PE-centric work units ----
            fill_q = deque()

            def fill(n):
                for _ in range(n):
                    if not fill_q:
                        return
                    fill_q.popleft()()

            def fill_all():
                while fill_q:
                    fill_q.popleft()()

            def enq_vproj(c):
                """V projection of chunk c: 8 units of 4 matmuls + 2 copies."""
                if c >= NCHUNK:
                    return
                bc, c4 = divmod(c, 4)
                batch_tiles(bc)
                xt, vt = xt_of.pop(c), [None]

                def unit(tt, h):
                    def emit():
                        if vt[0] is None:
                            vt[0] = wkp.tile([128, 512], F32, tag="wk",
                                             name=f"vt{c}")
                        for it in range(4 * h, 4 * h + 4):
                            nc.tensor.matmul(vt[0][:, 128 * tt:128 * tt + 128],
                                             xt[:, it, 128 * tt:128 * tt + 128],
                                             wv_sb[:, it, :],
                                             start=(it == 0), stop=(it == 7))
                        if tt == 3 and h == 1:
                            vv = vt[0].rearrange("p (t c) -> p t c", t=4)
                            nc.vector.tensor_copy(
                                v_of[bc][:, 4 * c4:4 * c4 + 4, 0:64], vv[:, :, 0:64])
                            nc.vector.tensor_copy(
                                v_of[bc][:, 4 * c4:4 * c4 + 4, 65:129], vv[:, :, 64:128])
                    return emit
                for tt in range(4):
                    for h in range(2):
                        fill_q.append(unit(tt, h))

            def enq_qkproj(c):
                """Q/K projection + rope of chunk c: units of 4 matmuls."""
                if c >= NCHUNK:
                    return
                bc, c4 = divmod(c, 4)
                batch_tiles(bc)
                xt = xt_of[c]
                t0 = 512 * c4
                state = {}

                def mm_unit(w_sb, key, h):
                    def emit():
                        if key not in state:
                            state[key] = wkp.tile([128, 512], F32, tag="wk",
                                                  name=f"{key}{c}")
                        ps = state[key]
                        for it in range(4 * h, 4 * h + 4):
                            nc.tensor.matmul(ps, w_sb[:, it, :], xt[:, it, :],
                                             start=(it == 0), stop=(it == 7))
                    return emit

                def rope_unit(key, dst):
                    def emit():
                        ps = state[key]
                        tmp2 = ropep.tile([128, 512], BF16, tag="rtmp2",
                                          name=f"t2{c}{key}")
                        if USE_SHUFFLE:
                            swf = ropep.tile([128, 512], F32, tag="rsw",
                                             name=f"sw{c}{key}")
                            nc.vector.stream_shuffle(swf, ps, SHUF_MASK)
                            nc.vector.tensor_mul(tmp2, ps, cos_sb[:, t0:t0 + 512])
                            sw_src = swf
                        else:
                            sb_c = ropep.tile([128, 512], BF16, tag="rcopy",
                                              name=f"sbc{c}{key}")
                            nc.vector.tensor_copy(sb_c, ps)
                            sw_ps = wkp.tile([128, 512], F32, tag="wk",
                                             name=f"sw{c}{key}")
                            nc.tensor.matmul(sw_ps, psw_sb, sb_c,
                                             start=True, stop=True)
                            nc.vector.tensor_mul(tmp2, sb_c, cos_sb[:, t0:t0 + 512])
                            sw_src = sw_ps
                        tmpf = ropep.tile([128, 512],
                                          BF16 if POOL_ROPE_ADD else F32,
                                          tag="rtmp", name=f"tf{c}{key}")
                        nc.vector.tensor_mul(tmpf, sw_src, sin_sb[:, t0:t0 + 512])
                        if POOL_ROPE_ADD:
                            nc.gpsimd.tensor_add(dst[:, t0:t0 + 512], tmp2, tmpf)
                        else:
                            nc.vector.tensor_add(dst[:, t0:t0 + 512], tmp2, tmpf)
                    return emit

                fill_q.append(mm_unit(wq_sb, "qt", 0))
                fill_q.append(mm_unit(wq_sb, "qt", 1))
                fill_q.append(rope_unit("qt", qtr_of[bc]))
                fill_q.append(mm_unit(wk_sb, "kt", 0))
                fill_q.append(mm_unit(wk_sb, "kt", 1))
                fill_q.append(rope_unit("kt", ktr_of[bc]))

            def enq_finish(bi, tt):
                """transpose + o_proj + store for token tile tt of batch bi."""
                tb0 = (bi % B) * S
                state = {}

                def tr_unit():
                    tp_ps = pvp.tile([128, 128], BF16, tag="pv",
                                     name=f"tp{bi}_{tt}")
                    nc.tensor.matmul(tp_ps, o_of[bi][:, tt, :], id_sb,
                                     is_transpose=True)
                    ot_t = otp.tile([128, 128], BF16, tag="ot",
                                    name=f"ot{bi}_{tt}")
                    nc.vector.tensor_copy(ot_t, tp_ps)
                    state["ot"] = ot_t

                def op_unit(oc):
                    def emit():
                        op_ps = pvp.tile([128, 512], F32, tag="pv",
                                         name=f"op{bi}_{tt}_{oc}")
                        nc.tensor.matmul(op_ps, state["ot"],
                                         wot_sb[:, 512 * oc:512 * oc + 512],
                                         start=True, stop=True)
                        if "yo" not in state:
                            state["yo"] = yop.tile([128, 1024], BF16, tag="yo",
                                                   name=f"yo{bi}_{tt}")
                        yo = state["yo"]
                        if (2 * tt + oc) % 3 == 0:
                            nc.scalar.copy(yo[:, 512 * oc:512 * oc + 512], op_ps)
                        else:
                            nc.vector.tensor_copy(yo[:, 512 * oc:512 * oc + 512],
                                                  op_ps)
                        if oc == 1:
                            nc.sync.dma_start(
                                out=y[tb0 + 128 * tt:tb0 + 128 * tt + 128, :],
                                in_=yo)
                    return emit

                fill_q.append(tr_unit)
                fill_q.append(op_unit(0))
                fill_q.append(op_unit(1))

            # ---- attention step for query chunk g ----
            def attn_step(g):
                bi, qc = divmod(g, 4)
                q0 = 512 * qc
                qtr, ktr, v_sb = qtr_of[bi], ktr_of[bi], v_of[bi]
                o_sb, recip_sb = o_of[bi], rc_of[bi]
                nkp = 2 * qc + 2
                p_cur = {}
                pv_t = {}

                def emit_scores(kp, hh_list=(0, 1)):
                    for hh in hh_list:
                        h0 = 64 * hh
                        st_ps = stp.tile([128, 1024], F32, tag="st",
                                         name=f"st{g}_{kp}_{hh}")
                        p_t = pp.tile([128, 1024], BF16, tag="p",
                                      name=f"p{g}_{kp}_{hh}")
                        if kp < 2 * qc:
                            for j in range(2):
                                ki = 2 * kp + j
                                nc.tensor.matmul(
                                    st_ps[:, 512 * j:512 * j + 512],
                                    ktr[h0:h0 + 64, 128 * ki:128 * ki + 128],
                                    qtr[h0:h0 + 64, q0:q0 + 512],
                                    start=True, stop=True)
                            nc.scalar.activation(p_t, st_ps, AF.Exp, scale=0.125)
                        else:
                            jbase = 2 * (kp - 2 * qc)
                            for jj in range(2):
                                j = jbase + jj
                                ki = 4 * qc + j
                                c0 = 512 * jj + 128 * j
                                nc.tensor.matmul(
                                    st_ps[:, c0:512 * jj + 512],
                                    ktr[h0:h0 + 64, 128 * ki:128 * ki + 128],
                                    qtr[h0:h0 + 64, q0 + 128 * j:q0 + 512],
                                    start=True, stop=True)
                                nc.scalar.activation(
                                    p_t[:, c0:512 * jj + 512],
                                    st_ps[:, c0:512 * jj + 512],
                                    AF.Exp, scale=0.125)
                                if POOL_MASK:
                                    nc.gpsimd.tensor_mul(
                                        p_t[:, c0:c0 + 128],
                                        p_t[:, c0:c0 + 128], tri_sb)
                                else:
                                    nc.vector.tensor_mul(
                                        p_t[:, c0:c0 + 128],
                                        p_t[:, c0:c0 + 128], tri_sb)
                        p_cur[(kp, hh)] = p_t

                done_cnt = {}

                def chain_unit(qt_l, hh):
                    # one complete softmax-weighted V accumulation for one
                    # (query tile, head): a single start->stop PSUM group so
                    # the 2KB zero-region never holds two open groups.
                    def emit():
                        qt = 4 * qc + qt_l
                        nki = 4 * qc + qt_l + 1
                        pv_ps = pvp.tile([128, 65], F32, tag="pv",
                                         name=f"pv{g}_{qt_l}_{hh}")
                        for ki in range(nki):
                            jj = ki % 2
                            nc.tensor.matmul(
                                pv_ps,
                                p_cur[(ki // 2, hh)][:, 512 * jj + 128 * qt_l:
                                                     512 * jj + 128 * qt_l + 128],
                                v_sb[:, ki, 65 * hh:65 * hh + 65],
                                start=(ki == 0), stop=(ki == nki - 1))
                        rcol = 2 * qt + hh
                        nc.vector.reciprocal(recip_sb[:, rcol:rcol + 1],
                                             pv_ps[:, 64:65])
                        nc.vector.tensor_scalar_mul(
                            o_sb[:, qt, 64 * hh:64 * hh + 64],
                            pv_ps[:, 0:64],
                            recip_sb[:, rcol:rcol + 1])
                        done_cnt[qt_l] = done_cnt.get(qt_l, 0) + 1
                        if done_cnt[qt_l] == 2:
                            enq_finish(bi, qt)
                    return emit

                # beats: scores(kp) with fill between; PV chains are enqueued
                # as fill units once their last score tile has been emitted
                n_beats = nkp
                fill_per = max(1, (len(fill_q) + 11 + n_beats - 1) // n_beats)
                for kp in range(nkp):
                    emit_scores(kp, (0,))
                    fill(fill_per - fill_per // 2)
                    emit_scores(kp, (1,))
                    if kp == 2 * qc:
                        for qt_l in (0, 1):
                            for hh in range(2):
                                fill_q.append(chain_unit(qt_l, hh))
                    elif kp == 2 * qc + 1:
                        for qt_l in (2, 3):
                            for hh in range(2):
                                fill_q.append(chain_unit(qt_l, hh))
                    fill(fill_per // 2)

            # ---- prologue ----
            enq_qkproj(0)
            fill_all()

            # ---- steady-state steps ----
            for g in range(NCHUNK):
                xt_prefetch(g + 3)
                enq_vproj(g)
                enq_qkproj(g + 1)
                attn_step(g)

            fill_all()

    nc.compile()
    return nc


_NC_CACHE = {}


def _get_nc(nb=B):
    if nb not in _NC_CACHE:
        _NC_CACHE[nb] = build_nc(nb)
    return _NC_CACHE[nb]


USE_SHUFFLE_HOST = USE_SHUFFLE


def _pmajor(wt):
    # [1024, 128] -> [128, 1024] with element [p, 128*it + f] = wt[128*it + p, f]
    return np.ascontiguousarray(
        wt.reshape(8, 128, 128).transpose(1, 0, 2).reshape(128, 1024)).astype(BFNP)


def _host_prep(x, Wq, Wk, Wv, Wo):
    x2 = np.ascontiguousarray(
        x.reshape(BS, 8, 128).transpose(2, 1, 0)).astype(BFNP)  # [128, 8, BS]

    half = 32
    inv_freq = 1.0 / (ROPE_THETA ** (np.arange(half, dtype=np.float64) / half))
    freqs = np.arange(S, dtype=np.float64)[:, None] * inv_freq[None, :]
    c_ = np.cos(freqs).astype(np.float32).T      # [32, S]
    s_ = np.sin(freqs).astype(np.float32).T
    if USE_SHUFFLE:
        # row r of each 64-block uses freq j(r); sign -s for even-dim rows
        cosb = np.ascontiguousarray(np.vstack(
            [c_[0:16], c_[0:16], c_[16:32], c_[16:32]] * 2)).astype(BFNP)
        sinf = np.ascontiguousarray(np.vstack(
            [-s_[0:16], s_[0:16], -s_[16:32], s_[16:32]] * 2)).astype(BFNP)
    else:
        cosb = np.ascontiguousarray(np.tile(c_, (4, 1))).astype(BFNP)
        sinf = np.ascontiguousarray(np.vstack([-s_, s_, -s_, s_])).astype(BFNP)

    perm = np.zeros(128, dtype=np.int64)
    partner = np.zeros(128, dtype=np.int64)
    for hh in range(2):
        for r in range(64):
            if USE_SHUFFLE:
                # pairs j: rows [0:16)=e(0..15) [16:32)=o(0..15)
                #          [32:48)=e(16..31) [48:64)=o(16..31)
                if r < 16:
                    d = 2 * r
                elif r < 32:
                    d = 2 * (r - 16) + 1
                elif r < 48:
                    d = 2 * (r - 16)
                else:
                    d = 2 * (r - 32) + 1
                perm[64 * hh + r] = 64 * hh + d
                partner[64 * hh + r] = 64 * hh + (r // 32) * 32 + (r + 16) % 32
            else:
                perm[64 * hh + r] = 64 * hh + (2 * r if r < 32 else 2 * (r - 32) + 1)
                partner[64 * hh + r] = 64 * hh + (r + 32) % 64
    pswap = np.zeros((128, 128), dtype=np.float32)
    pswap[partner, np.arange(128)] = 1.0
    pswap = pswap.astype(BFNP)

    # tri[k, q] = 1 if q >= k (keys attend up to and including the query pos)
    tri = (np.arange(128)[None, :] >= np.arange(128)[:, None]).astype(BFNP)
    iden = np.eye(128, dtype=np.float32).astype(BFNP)

    in_maps = []
    for c in range(NCORES):
        sl = slice(128 * c, 128 * c + 128)
        in_maps.append({
            "xb": x2,
            "wqt": _pmajor(Wq[sl][perm].T),
            "wkt": _pmajor(Wk[sl][perm].T),
            "wvt": _pmajor(Wv[sl].T),
            "wot": np.ascontiguousarray(Wo[:, sl].T).astype(BFNP),
            "cosb": cosb,
            "sinf": sinf,
            "pswap": pswap,
            "tri": tri,
            "iden": iden,
        })
    return in_maps


def kernel(x, Wq, Wk, Wv, Wo):
    global LAST_RESULTS
    x = np.asarray(x, dtype=np.float32)
    Wq = np.asarray(Wq, dtype=np.float32)
    Wk = np.asarray(Wk, dtype=np.float32)
    Wv = np.asarray(Wv, dtype=np.float32)
    Wo = np.asarray(Wo, dtype=np.float32)

    nc = _get_nc(B)
    in_maps = _host_prep(x, Wq, Wk, Wv, Wo)
    res = run_bass_kernel_spmd(nc, in_maps, core_ids=list(range(NCORES)),
                               trace=TRACE)
    LAST_RESULTS = res
    out = np.zeros((BS, D), dtype=np.float32)
    for c in range(NCORES):
        out += np.asarray(res.results[c]["y"]).astype(np.float32)
    return out.reshape(B, S, D)


# revision 101
# speedup vs baseline: 1.0034x; 1.0034x over previous
"""Causal MHA with RoPE on 8 Trainium2 NeuronCores.

Sharding: tensor-parallel over heads. Core c owns heads {2c, 2c+1} (a 128-wide
slice of the model dim). Each core computes Q/K/V projections for its heads,
full causal attention, and a partial o_proj; the host sums the 8 partial
outputs (the "all-reduce").

Schedule: a software-pipelined flat stream of "steps", one per (batch, query
chunk), emitted so the in-order PE stream rarely parks on a dependency:
  - x is pre-transposed on the host ([128, 8, BS] bf16); chunk g+3 prefetched
  - V projection of chunk g and Q/K projection + RoPE of chunk g+1 are
    emitted as small matmul units through a fill queue
  - RoPE uses a DVE stream_shuffle (pair partner = +16 within each 32-row
    quadrant, set up by the host-side head-dim permutation) + cos/sin
    multiplies, with the final add on gpsimd
  - attention for chunk g: per 256-key tile: scores (PE, causal-trimmed at
    128-col granularity) -> exp (ACT) -> diagonal triangle mask (gpsimd)
  - per (query tile, head): one PV chain unit — a single start->stop PSUM
    accumulation (one open group per 2KB PSUM bank at a time!) in [q, d]
    form with a ones column for the softmax denominator, then reciprocal
    (DVE) and a normalize fused into the PSUM->SBUF copy (tensor_scalar_mul)
  - per finished query tile: PE-transpose of O, o_proj, bf16 y store; these
    units are interleaved as further PE fill
PSUM budget (8 banks): scores 2x[128,1024] (4) + proj work ring (2) +
pv-chain/transpose/o_proj ring (2).
"""
import sys
sys.path.insert(0, '/opt/trn_rl_repo')

from collections import deque

import numpy as np
import ml_dtypes

import concourse.bass as bass
from concourse import bacc
import concourse.mybir as mybir
import concourse.tile as tile
from concourse.bass_utils import run_bass_kernel_spmd

BFNP = ml_dtypes.bfloat16
F32 = mybir.dt.float32
BF16 = mybir.dt.bfloat16
AF = mybir.ActivationFunctionType
ALU = mybir.AluOpType

B, S, D = 4, 2048, 1024
NCORES = 8
BS = B * S
ROPE_THETA = 10000.0

# engine knobs (fallbacks if gpsimd rejects tensor ops: set to False)
POOL_MASK = True
POOL_ROPE_ADD = True
USE_SHUFFLE = True
DVE_DIVIDE = False
SHUF_MASK = [(i + 16) % 32 for i in range(32)]

TRACE = False
LAST_RESULTS = None


def build_nc(nb=B, debug=False):
    nc = bacc.Bacc()
    xb = nc.dram_tensor("xb", [128, 8, BS], BF16, kind="ExternalInput")
    wqt = nc.dram_tensor("wqt", [128, D], BF16, kind="ExternalInput")
    wkt = nc.dram_tensor("wkt", [128, D], BF16, kind="ExternalInput")
    wvt = nc.dram_tensor("wvt", [128, D], BF16, kind="ExternalInput")
    wot = nc.dram_tensor("wot", [128, D], BF16, kind="ExternalInput")
    cosb = nc.dram_tensor("cosb", [128, S], BF16, kind="ExternalInput")
    sinf = nc.dram_tensor("sinf", [128, S], BF16, kind="ExternalInput")
    pswap = nc.dram_tensor("pswap", [128, 128], BF16, kind="ExternalInput")
    tri = nc.dram_tensor("tri", [128, 128], BF16, kind="ExternalInput")
    iden = nc.dram_tensor("iden", [128, 128], BF16, kind="ExternalInput")
    y = nc.dram_tensor("y", [BS, D], BF16, kind="ExternalOutput")

    NCHUNK = 4 * nb

    with tile.TileContext(nc) as tc:
        with tc.tile_pool(name="const", bufs=1) as constp, \
             tc.tile_pool(name="xt", bufs=4) as xtp, \
             tc.tile_pool(name="qk", bufs=2) as qkp, \
             tc.tile_pool(name="vsb", bufs=2) as vsp, \
             tc.tile_pool(name="rope", bufs=6) as ropep, \
             tc.tile_pool(name="ptile", bufs=28) as pp, \
             tc.tile_pool(name="osb", bufs=2) as osp, \
             tc.tile_pool(name="rc", bufs=2) as rcp, \
             tc.tile_pool(name="ott", bufs=4) as otp, \
             tc.tile_pool(name="yout", bufs=6) as yop, \
             tc.tile_pool(name="wk", bufs=2, space="PSUM") as wkp, \
             tc.tile_pool(name="stp", bufs=2, space="PSUM") as stp, \
             tc.tile_pool(name="pvp", bufs=2, space="PSUM") as pvp:
            xt_of = {}

            def xt_prefetch(c):
                if c < NCHUNK and c not in xt_of:
                    bc, c4 = divmod(c, 4)
                    t0 = (bc % B) * S + 512 * c4
                    xt = xtp.tile([128, 8, 512], BF16, tag="xt", name=f"xt{c}")
                    nc.sync.dma_start(out=xt, in_=xb[:, :, t0:t0 + 512])
                    xt_of[c] = xt

            xt_prefetch(0)
            wtile = constp.tile([128, 2], F32)
            nc.gpsimd.memset(wtile, 0.0)
            warm = constp.tile([128, 2], F32)
            nc.scalar.activation(warm, wtile, AF.Exp)
            # PE pstate warm-up: zero matmuls while the first x chunk loads
            wz = constp.tile([128, 128], BF16)
            nc.gpsimd.memset(wz, 0.0)
            wmm = wkp.tile([128, 128], F32, tag="wk", name="wmm")
            for _ in range(48):
                nc.tensor.matmul(wmm, wz, wz, start=True, stop=True)
            wq_sb = constp.tile([128, 8, 128], BF16)
            nc.scalar.dma_start(out=wq_sb, in_=wqt.rearrange("p (a f) -> p a f", a=8))
            wk_sb = constp.tile([128, 8, 128], BF16)
            nc.scalar.dma_start(out=wk_sb, in_=wkt.rearrange("p (a f) -> p a f", a=8))
            xt_prefetch(1)
            xt_prefetch(2)
            wv_sb = constp.tile([128, 8, 128], BF16)
            nc.scalar.dma_start(out=wv_sb, in_=wvt.rearrange("p (a f) -> p a f", a=8))
            wot_sb = constp.tile([128, D], BF16)
            nc.scalar.dma_start(out=wot_sb, in_=wot[:, :])
            cos_sb = constp.tile([128, S], BF16)
            nc.scalar.dma_start(out=cos_sb, in_=cosb[:, :])
            sin_sb = constp.tile([128, S], BF16)
            nc.scalar.dma_start(out=sin_sb, in_=sinf[:, :])
            if not USE_SHUFFLE:
                psw_sb = constp.tile([128, 128], BF16)
                nc.gpsimd.dma_start(out=psw_sb, in_=pswap[:, :])
            tri_sb = constp.tile([128, 128], BF16)
            nc.gpsimd.dma_start(out=tri_sb, in_=tri[:, :])
            id_sb = constp.tile([128, 128], BF16)
            nc.gpsimd.dma_start(out=id_sb, in_=iden[:, :])
            # ---- per-batch SBUF tile state ----
            qtr_of, ktr_of, v_of, o_of, rc_of = {}, {}, {}, {}, {}

            def batch_tiles(bi):
                if bi not in qtr_of:
                    qtr_of[bi] = qkp.tile([128, S], BF16, tag="qtr",
                                          name=f"qtr{bi}")
                    ktr_of[bi] = qkp.tile([128, S], BF16, tag="ktr",
                                          name=f"ktr{bi}")
                    v_of[bi] = vsp.tile([128, 16, 130], BF16, tag="v",
                                        name=f"v{bi}")
                    nc.gpsimd.memset(v_of[bi][:, :, 64:65], 1.0)
                    nc.gpsimd.memset(v_of[bi][:, :, 129:130], 1.0)
                    o_of[bi] = osp.tile([128, 16, 128], BF16, tag="o",
                                        name=f"o{bi}")
                    rc_of[bi] = rcp.tile([128, 32], F32, tag="rcp",
                                         name=f"rc{bi}")

            # ---- fill queue: callables emitting PE-centric work units ----
            fill_q = deque()

            def fill(n):
                for _ in range(n):
                    if not fill_q:
                        return
                    fill_q.popleft()()

            def fill_all():
                while fill_q:
                    fill_q.popleft()()

            def enq_vproj(c):
                """V projection of chunk c: 8 units of 4 matmuls + 2 copies."""
                if c >= NCHUNK:
                    return
                bc, c4 = divmod(c, 4)
                batch_tiles(bc)
                xt, vt = xt_of.pop(c), [None]

                def unit(tt, h):
                    def emit():
                        if vt[0] is None:
                            vt[0] = wkp.tile([128, 512], F32, tag="wk",
                                             name=f"vt{c}")
                        for it in range(4 * h, 4 * h + 4):
                            nc.tensor.matmul(vt[0][:, 128 * tt:128 * tt + 128],
                                             xt[:, it, 128 * tt:128 * tt + 128],
                                             wv_sb[:, it, :],
                                             start=(it == 0), stop=(it == 7))
                        if tt == 3 and h == 1:
                            vv = vt[0].rearrange("p (t c) -> p t c", t=4)
                            nc.vector.tensor_copy(
                                v_of[bc][:, 4 * c4:4 * c4 + 4, 0:64], vv[:, :, 0:64])
                            nc.vector.tensor_copy(
                                v_of[bc][:, 4 * c4:4 * c4 + 4, 65:129], vv[:, :, 64:128])
                    return emit
                for tt in range(4):
                    for h in range(2):
                        fill_q.append(unit(tt, h))

            def enq_qkproj(c):
                """Q/K projection + rope of chunk c: units of 4 matmuls."""
                if c >= NCHUNK:
                    return
                bc, c4 = divmod(c, 4)
                batch_tiles(bc)
                xt = xt_of[c]
                t0 = 512 * c4
                state = {}

                def mm_unit(w_sb, key, h):
                    def emit():
                        if key not in state:
                            state[key] = wkp.tile([128, 512], F32, tag="wk",
                                                  name=f"{key}{c}")
                        ps = state[key]
                        for it in range(4 * h, 4 * h + 4):
                            nc.tensor.matmul(ps, w_sb[:, it, :], xt[:, it, :],
                                             start=(it == 0), stop=(it == 7))
                    return emit

                def rope_unit(key, dst):
                    def emit():
                        ps = state[key]
                        tmp2 = ropep.tile([128, 512], BF16, tag="rtmp2",
                                          name=f"t2{c}{key}")
                        if USE_SHUFFLE:
                            swf = ropep.tile([128, 512], F32, tag="rsw",
                                             name=f"sw{c}{key}")
                            nc.vector.stream_shuffle(swf, ps, SHUF_MASK)
                            nc.vector.tensor_mul(tmp2, ps, cos_sb[:, t0:t0 + 512])
                            sw_src = swf
                        else:
                            sb_c = ropep.tile([128, 512], BF16, tag="rcopy",
                                              name=f"sbc{c}{key}")
                            nc.vector.tensor_copy(sb_c, ps)
                            sw_ps = wkp.tile([128, 512], F32, tag="wk",
                                             name=f"sw{c}{key}")
                            nc.tensor.matmul(sw_ps, psw_sb, sb_c,
                                             start=True, stop=True)
                            nc.vector.tensor_mul(tmp2, sb_c, cos_sb[:, t0:t0 + 512])
                            sw_src = sw_ps
                        tmpf = ropep.tile([128, 512],
                                          BF16 if POOL_ROPE_ADD else F32,
                                          tag="rtmp", name=f"tf{c}{key}")
                        nc.vector.tensor_mul(tmpf, sw_src, sin_sb[:, t0:t0 + 512])
                        if POOL_ROPE_ADD:
                            nc.gpsimd.tensor_add(dst[:, t0:t0 + 512], tmp2, tmpf)
                        else:
                            nc.vector.tensor_add(dst[:, t0:t0 + 512], tmp2, tmpf)
                    return emit

                fill_q.append(mm_unit(wq_sb, "qt", 0))
                fill_q.append(mm_unit(wq_sb, "qt", 1))
                fill_q.append(rope_unit("qt", qtr_of[bc]))
                fill_q.append(mm_unit(wk_sb, "kt", 0))
                fill_q.append(mm_unit(wk_sb, "kt", 1))
                fill_q.append(rope_unit("kt", ktr_of[bc]))

            def enq_finish(bi, tt):
                """transpose + o_proj + store for token tile tt of batch bi."""
                tb0 = (bi % B) * S
                state = {}

                def tr_unit():
                    tp_ps = pvp.tile([128, 128], BF16, tag="pv",
                                     name=f"tp{bi}_{tt}")
                    nc.tensor.matmul(tp_ps, o_of[bi][:, tt, :], id_sb,
                                     is_transpose=True)
                    ot_t = otp.tile([128, 128], BF16, tag="ot",
                                    name=f"ot{bi}_{tt}")
                    nc.vector.tensor_copy(ot_t, tp_ps)
                    state["ot"] = ot_t

                def op_unit(oc):
                    def emit():
                        op_ps = pvp.tile([128, 512], F32, tag="pv",
                                         name=f"op{bi}_{tt}_{oc}")
                        nc.tensor.matmul(op_ps, state["ot"],
                                         wot_sb[:, 512 * oc:512 * oc + 512],
                                         start=True, stop=True)
                        if "yo" not in state:
                            state["yo"] = yop.tile([128, 1024], BF16, tag="yo",
                                                   name=f"yo{bi}_{tt}")
                        yo = state["yo"]
                        if (2 * tt + oc) % 3 == 0:
                            nc.scalar.copy(yo[:, 512 * oc:512 * oc + 512], op_ps)
                        else:
                            nc.vector.tensor_copy(yo[:, 512 * oc:512 * oc + 512],
                                                  op_ps)
                        if oc == 1:
                            nc.sync.dma_start(
                                out=y[tb0 + 128 * tt:tb0 + 128 * tt + 128, :],
                                in_=yo)
                    return emit

                fill_q.append(tr_unit)
                fill_q.append(op_unit(0))
                fill_q.append(op_unit(1))

            # ---- attention step for query chunk g ----
            def attn_step(g):
                bi, qc = divmod(g, 4)
                q0 = 512 * qc
                qtr, ktr, v_sb = qtr_of[bi], ktr_of[bi], v_of[bi]
                o_sb, recip_sb = o_of[bi], rc_of[bi]
                nkp = 2 * qc + 2
                p_cur = {}
                pv_t = {}

                def emit_scores(kp, hh_list=(0, 1)):
                    for hh in hh_list:
                        h0 = 64 * hh
                        st_ps = stp.tile([128, 1024], F32, tag="st",
                                         name=f"st{g}_{kp}_{hh}")
                        p_t = pp.tile([128, 1024], BF16, tag="p",
                                      name=f"p{g}_{kp}_{hh}")
                        if kp < 2 * qc:
                            for j in range(2):
                                ki = 2 * kp + j
                                nc.tensor.matmul(
                                    st_ps[:, 512 * j:512 * j + 512],
                                    ktr[h0:h0 + 64, 128 * ki:128 * ki + 128],
                                    qtr[h0:h0 + 64, q0:q0 + 512],
                                    start=True, stop=True)
                            nc.scalar.activation(p_t, st_ps, AF.Exp, scale=0.125)
                        else:
                            jbase = 2 * (kp - 2 * qc)
                            for jj in range(2):
                                j = jbase + jj
                                ki = 4 * qc + j
                                c0 = 512 * jj + 128 * j
                                nc.tensor.matmul(
                                    st_ps[:, c0:512 * jj + 512],
                                    ktr[h0:h0 + 64, 128 * ki:128 * ki + 128],
                                    qtr[h0:h0 + 64, q0 + 128 * j:q0 + 512],
                                    start=True, stop=True)
                                nc.scalar.activation(
                                    p_t[:, c0:512 * jj + 512],
                                    st_ps[:, c0:512 * jj + 512],
                                    AF.Exp, scale=0.125)
                                if POOL_MASK:
                                    nc.gpsimd.tensor_mul(
                                        p_t[:, c0:c0 + 128],
                                        p_t[:, c0:c0 + 128], tri_sb)
                                else:
                                    nc.vector.tensor_mul(
                                        p_t[:, c0:c0 + 128],
                                        p_t[:, c0:c0 + 128], tri_sb)
                        p_cur[(kp, hh)] = p_t

                done_cnt = {}

                def chain_unit(qt_l, hh):
                    # one complete softmax-weighted V accumulation for one
                    # (query tile, head): a single start->stop PSUM group so
                    # the 2KB zero-region never holds two open groups.
                    def emit():
                        qt = 4 * qc + qt_l
                        nki = 4 * qc + qt_l + 1
                        pv_ps = pvp.tile([128, 65], F32, tag="pv",
                                         name=f"pv{g}_{qt_l}_{hh}")
                        for ki in range(nki):
                            jj = ki % 2
                            nc.tensor.matmul(
                                pv_ps,
                                p_cur[(ki // 2, hh)][:, 512 * jj + 128 * qt_l:
                                                     512 * jj + 128 * qt_l + 128],
                                v_sb[:, ki, 65 * hh:65 * hh + 65],
                                start=(ki == 0), stop=(ki == nki - 1))
                        rcol = 2 * qt + hh
                        nc.vector.reciprocal(recip_sb[:, rcol:rcol + 1],
                                             pv_ps[:, 64:65])
                        nc.vector.tensor_scalar_mul(
                            o_sb[:, qt, 64 * hh:64 * hh + 64],
                            pv_ps[:, 0:64],
                            recip_sb[:, rcol:rcol + 1])
                        done_cnt[qt_l] = done_cnt.get(qt_l, 0) + 1
                        if done_cnt[qt_l] == 2:
                            enq_finish(bi, qt)
                    return emit

                # beats: scores(kp) with fill between; PV chains are enqueued
                # as fill units once their last score tile has been emitted
                n_beats = nkp
                fill_per = max(1, (len(fill_q) + 11 + n_beats - 1) // n_beats)
                for kp in range(nkp):
                    emit_scores(kp, (0,))
                    fill(fill_per)
                    emit_scores(kp, (1,))
                    if kp == 2 * qc:
                        for qt_l in (0, 1):
                            for hh in range(2):
                                fill_q.append(chain_unit(qt_l, hh))
                    elif kp == 2 * qc + 1:
                        for qt_l in (2, 3):
                            for hh in range(2):
                                fill_q.append(chain_unit(qt_l, hh))

            # ---- prologue ----
            enq_qkproj(0)
            fill_all()

            # ---- steady-state steps ----
            for g in range(NCHUNK):
                xt_prefetch(g + 3)
                enq_vproj(g)
                enq_qkproj(g + 1)
                attn_step(g)

            fill_all()

    nc.compile()
    return nc


_NC_CACHE = {}


def _get_nc(nb=B):
    if nb not in _NC_CACHE:
        _NC_CACHE[nb] = build_nc(nb)
    return _NC_CACHE[nb]


USE_SHUFFLE_HOST = USE_SHUFFLE


def _pmajor(wt):
    # [1024, 128] -> [128, 1024] with element [p, 128*it + f] = wt[128*it + p, f]
    return np.ascontiguousarray(
        wt.reshape(8, 128, 128).transpose(1, 0, 2).reshape(128, 1024)).astype(BFNP)


def _host_prep(x, Wq, Wk, Wv, Wo):
    x2 = np.ascontiguousarray(
        x.reshape(BS, 8, 128).transpose(2, 1, 0)).astype(BFNP)  # [128, 8, BS]

    half = 32
    inv_freq = 1.0 / (ROPE_THETA ** (np.arange(half, dtype=np.float64) / half))
    freqs = np.arange(S, dtype=np.float64)[:, None] * inv_freq[None, :]
    c_ = np.cos(freqs).astype(np.float32).T      # [32, S]
    s_ = np.sin(freqs).astype(np.float32).T
    if USE_SHUFFLE:
        # row r of each 64-block uses freq j(r); sign -s for even-dim rows
        cosb = np.ascontiguousarray(np.vstack(
            [c_[0:16], c_[0:16], c_[16:32], c_[16:32]] * 2)).astype(BFNP)
        sinf = np.ascontiguousarray(np.vstack(
            [-s_[0:16], s_[0:16], -s_[16:32], s_[16:32]] * 2)).astype(BFNP)
    else:
        cosb = np.ascontiguousarray(np.tile(c_, (4, 1))).astype(BFNP)
        sinf = np.ascontiguousarray(np.vstack([-s_, s_, -s_, s_])).astype(BFNP)

    perm = np.zeros(128, dtype=np.int64)
    partner = np.zeros(128, dtype=np.int64)
    for hh in range(2):
        for r in range(64):
            if USE_SHUFFLE:
                # pairs j: rows [0:16)=e(0..15) [16:32)=o(0..15)
                #          [32:48)=e(16..31) [48:64)=o(16..31)
                if r < 16:
                    d = 2 * r
                elif r < 32:
                    d = 2 * (r - 16) + 1
                elif r < 48:
                    d = 2 * (r - 16)
                else:
                    d = 2 * (r - 32) + 1
                perm[64 * hh + r] = 64 * hh + d
                partner[64 * hh + r] = 64 * hh + (r // 32) * 32 + (r + 16) % 32
            else:
                perm[64 * hh + r] = 64 * hh + (2 * r if r < 32 else 2 * (r - 32) + 1)
                partner[64 * hh + r] = 64 * hh + (r + 32) % 64
    pswap = np.zeros((128, 128), dtype=np.float32)
    pswap[partner, np.arange(128)] = 1.0
    pswap = pswap.astype(BFNP)

    # tri[k, q] = 1 if q >= k (keys attend up to and including the query pos)
    tri = (np.arange(128)[None, :] >= np.arange(128)[:, None]).astype(BFNP)
    iden = np.eye(128, dtype=np.float32).astype(BFNP)

    in_maps = []
    for c in range(NCORES):
        sl = slice(128 * c, 128 * c + 128)
        in_maps.append({
            "xb": x2,
            "wqt": _pmajor(Wq[sl][perm].T),
            "wkt": _pmajor(Wk[sl][perm].T),
            "wvt": _pmajor(Wv[sl].T),
            "wot": np.ascontiguousarray(Wo[:, sl].T).astype(BFNP),
            "cosb": cosb,
            "sinf": sinf,
            "pswap": pswap,
            "tri": tri,
            "iden": iden,
        })
    return in_maps


def kernel(x, Wq, Wk, Wv, Wo):
    global LAST_RESULTS
    x = np.asarray(x, dtype=np.float32)
    Wq = np.asarray(Wq, dtype=np.float32)
    Wk = np.asarray(Wk, dtype=np.float32)
    Wv = np.asarray(Wv, dtype=np.float32)
    Wo = np.asarray(Wo, dtype=np.float32)

    nc = _get_nc(B)
    in_maps = _host_prep(x, Wq, Wk, Wv, Wo)
    res = run_bass_kernel_spmd(nc, in_maps, core_ids=list(range(NCORES)),
                               trace=TRACE)
    LAST_RESULTS = res
    out = np.zeros((BS, D), dtype=np.float32)
    for c in range(NCORES):
        out += np.asarray(res.results[c]["y"]).astype(np.float32)
    return out.reshape(B, S, D)
